# revision 67
# baseline (speedup 1.0000x reference)
"""MLA (multi-latent attention) Trainium2 kernel, 8-core SPMD + collectives.

Sharding: tensor-parallel over heads (4 groups of 4 heads) x data-parallel
over batch (2), = 8 cores. The kv a-projection (x@Wkva -> ckv, k_pe) is
token-sharded: core (be, hg) computes tokens [hg*512,(hg+1)*512) of batch be,
then an AllGather over each 4-core batch group distributes the full ckv/k_pe.
The q a-projection (x@Wqa) is computed replicated -- its ~41us of extra
tensor work fills the ~70us first-collective rendezvous+transfer latency of
this stack, so the collective is fully hidden. q_b/kv_b output dims and
out_proj input dim are sharded by head. Each core returns a token-major
partial out-projection [n, 2048]; the host sums the 4 head-group partials
per batch element.

On-chip layout is feature-major ("T" = [features on partitions, tokens on
free]) so every matmul contracts over the partition dim with natural layouts.
Attention keeps keys on partitions / queries on free: scoresT = kfT.T @ qfT.
On causal-diagonal key blocks the -30000*U mask bias matmul is issued FIRST
(start=True, full query range) so the kn/kpe score matmuls can restrict
their free range to the unmasked queries. pT = exp(scale * scoresT),
out_avT = v-slices @ pT; softmax denominators accumulate on DVE. The
out-projection DMAs stream straight from PSUM to DRAM. No max subtraction
(logits are O(10), far from fp32 exp overflow).
"""

from contextlib import ExitStack

import numpy as np
import ml_dtypes

import concourse.bacc as bacc
import concourse.mybir as mybir
from concourse.tile import TileContext
from concourse import bass_isa, bass_utils

BF16 = mybir.dt.bfloat16
F32 = mybir.dt.float32
NPBF16 = ml_dtypes.bfloat16

EMBED = 2048
HEADS = 16
NOPE = 128
VDIM = 128
ROPE = 64
Q_HEAD = NOPE + ROPE  # 192
KV_RANK = 512
BASE = 10000.0
SCALE = 1.0 / float(np.sqrt(Q_HEAD))
MASK_BIAS = -30000.0

NH = 4          # heads per core
KC = EMBED // 128   # 16 k-chunks of the embedding dim
RC = KV_RANK // 128  # 4 k-chunks of the kv rank

GROUPS = [[0, 1, 2, 3], [4, 5, 6, 7]]
OWN = 512   # own-slice tokens: one full 512-token chunk per core

# psum bank budget: pss 3 + av 2 + pso 2 + den 1 = 8. den is dedicated so
# the per-head denominator matmul never WARs a score/out-proj bank behind
# its slow (DVE reciprocal) reader
_PS_TAGS = {"pss": 3, "av": 2, "pso": 2, "den": 1}
_PS_ROT = ["pss", "pss", "pss", "av", "av", "pso", "pso"]


def _emit(nc, n):
    """Trace the per-core kernel (same program on all 8 cores)."""
    TC = n // 512   # token chunks of 512
    NT = n // 128   # token chunks of 128
    AF = mybir.ActivationFunctionType
    OP = mybir.AluOpType

    # ---- DRAM I/O ----
    d_x = nc.dram_tensor("xc0", [128, KC, 512], BF16, kind="ExternalInput")
    d_xo = nc.dram_tensor("xown", [128, KC, OWN], BF16, kind="ExternalInput")
    d_wqa = nc.dram_tensor("wqa", [128, KC, 512], BF16, kind="ExternalInput")
    d_wkva = nc.dram_tensor("wkva", [128, KC, 576], BF16, kind="ExternalInput")
    d_wqb = nc.dram_tensor("wqb", [128, RC, NH * Q_HEAD], BF16, kind="ExternalInput")
    d_wk = nc.dram_tensor("wk", [128, RC, NH * NOPE], BF16, kind="ExternalInput")
    d_wv = nc.dram_tensor("wv", [128, RC, NH * VDIM], BF16, kind="ExternalInput")
    d_wout = nc.dram_tensor("wout", [128, NH, EMBED], BF16, kind="ExternalInput")
    d_cos = nc.dram_tensor("cosd", [128, n], BF16, kind="ExternalInput")
    d_sin = nc.dram_tensor("sind", [128, n], BF16, kind="ExternalInput")
    d_eye128 = nc.dram_tensor("eye128", [128, 128], BF16, kind="ExternalInput")
    d_trim = nc.dram_tensor("trimd", [128, 128], BF16, kind="ExternalInput")
    d_rotp = nc.dram_tensor("rotp", [128, 128], BF16, kind="ExternalInput")
    d_rotk = nc.dram_tensor("rotk", [64, 128], BF16, kind="ExternalInput")
    d_eyek = nc.dram_tensor("eyek", [64, 128], BF16, kind="ExternalInput")
    d_ones = nc.dram_tensor("onesd", [128, 1], BF16, kind="ExternalInput")
    d_out = nc.dram_tensor("out", [n, EMBED], BF16, kind="ExternalOutput")

    # ---- collective bounce buffers (DRAM, Internal) ----
    b_warm_in = nc.dram_tensor("bwin", [1, 64], BF16, kind="Internal")
    b_warm_out = nc.dram_tensor("bwout", [4, 64], BF16, kind="Internal")
    b_inB = nc.dram_tensor("binB", [128, 5, OWN], BF16, kind="Internal")
    b_outB = nc.dram_tensor("boutB", [4, 128, 5, OWN], BF16, kind="Internal")
    b_inQ = nc.dram_tensor("binQ", [128, 4, OWN], BF16, kind="Internal")
    b_outQ = nc.dram_tensor("boutQ", [4, 128, 4, OWN], BF16, kind="Internal")

    with TileContext(nc) as tc, ExitStack() as st:
        psum = st.enter_context(tc.tile_pool(name="psum", bufs=1, space="PSUM"))
        rot_i = [0]

        def ps_any(name):
            tag = _PS_ROT[rot_i[0] % len(_PS_ROT)]
            rot_i[0] += 1
            return psum.tile([128, 512], F32, tag=tag, bufs=_PS_TAGS[tag], name=name)

        def ps_tag(tag, name):
            return psum.tile([128, 512], F32, tag=tag, bufs=_PS_TAGS[tag], name=name)

        # warmup collective: starts the CC rendezvous clock at t~0. Its
        # payload is a scratch tensor; the result is unused.
        wp = st.enter_context(tc.tile_pool(name="warm", bufs=1))
        t_w = wp.tile([1, 64], BF16)
        nc.vector.memset(t_w, 0.0)
        nc.scalar.dma_start(out=b_warm_in.ap(), in_=t_w)
        nc.gpsimd.collective_compute(
            "AllGather", mybir.AluOpType.bypass, replica_groups=GROUPS,
            ins=[b_warm_in.ap()], outs=[b_warm_out.ap()],
        )

        # ---- mid pool: gathered phase-1 outputs + rope constants ----
        mid = st.enter_context(tc.tile_pool(name="mid", bufs=1))
        t_qa = mid.tile([128, RC, n], BF16)
        t_ckv = mid.tile([128, RC, n], BF16)
        t_kpr = mid.tile([64, n], BF16)  # raw k_pe (pre-rope)
        t_cos = mid.tile([128, n], BF16)
        t_sin = mid.tile([128, n], BF16)
        t_rotp = mid.tile([128, 128], BF16)
        t_rotk = mid.tile([64, 128], BF16)
        t_eyek = mid.tile([64, 128], BF16)
        # attention-phase constants + wout live here (NOT in attn_p): attn_p
        # aliases freed ph1 SBUF, which would gate their DMAs on the last
        # phase-1 reads (~90us) and stall attn(0)
        t_wout = mid.tile([128, NH, EMBED], BF16)
        t_eye128 = mid.tile([128, 128], BF16)
        t_trim = mid.tile([128, 128], BF16)
        t_ones = mid.tile([128, 1], BF16)
        # per-head q_pe, zero-padded to full 128 partitions: a 128-contract
        # matmul vs the duplicated t_kpe runs ~1.8x faster than 64-contract.
        # Lives in mid (not attn_p) so the upper-half memset can run at t~0
        # instead of waiting for the ph1 SBUF aliases to free
        t_qpe2 = mid.tile([128, NH, n], BF16)
        nc.vector.memset(t_qpe2[64:128], 0.0)

        # ---- phase-2 weights: prefetched during phase 1 ----
        ph2w = st.enter_context(tc.tile_pool(name="ph2w", bufs=1))
        t_wqb = ph2w.tile([128, RC, NH * Q_HEAD], BF16)
        t_wk = ph2w.tile([128, RC, NH * NOPE], BF16)
        t_wv = ph2w.tile([128, RC, NH * VDIM], BF16)

        # ===== phase 1: ckv own chunk (sharded) -> CC; qa all chunks =========
        with tc.tile_pool(name="ph1", bufs=1) as ph1:
            t_x = ph1.tile([128, KC, 512], BF16)
            t_xo = ph1.tile([128, KC, OWN], BF16)
            t_wqa = ph1.tile([128, KC, 512], BF16)
            t_wkva = ph1.tile([128, KC, 576], BF16)
            t_own = ph1.tile([128, 5, OWN], BF16)  # ckv 0:4, kpe [0:64,4]
            t_ownq = ph1.tile([128, 4, OWN], BF16)  # qa own chunk
            # critical path: xo (sync+gpsimd rings) + wqa (scalar) feed
            # qa-own; k-sub-splits let the matmul loops chase the DMAs
            for k4 in range(0, KC, 8):
                nc.sync.dma_start(
                    out=t_xo[:, k4 : k4 + 4], in_=d_xo.ap()[:, k4 : k4 + 4]
                )
                nc.gpsimd.dma_start(
                    out=t_xo[:, k4 + 4 : k4 + 8],
                    in_=d_xo.ap()[:, k4 + 4 : k4 + 8],
                )
            for k4 in range(0, KC, 4):
                nc.scalar.dma_start(
                    out=t_wqa[:, k4 : k4 + 4], in_=d_wqa.ap()[:, k4 : k4 + 4]
                )

            # x chunk 0 on gpsimd (needed ~40us in)
            for k4 in range(0, KC, 4):
                nc.scalar.dma_start(
                    out=t_wkva[:, k4 : k4 + 4], in_=d_wkva.ap()[:, k4 : k4 + 4]
                )
            for k4 in range(0, KC, 8):
                nc.gpsimd.dma_start(
                    out=t_x[:, k4 : k4 + 8], in_=d_x.ap()[:, k4 : k4 + 8]
                )
            nc.scalar.dma_start(out=t_wqb, in_=d_wqb.ap())
            nc.scalar.dma_start(out=t_wk, in_=d_wk.ap())
            nc.scalar.dma_start(out=t_wv, in_=d_wv.ap())
            nc.gpsimd.dma_start(out=t_cos, in_=d_cos.ap())
            nc.gpsimd.dma_start(out=t_sin, in_=d_sin.ap())
            nc.gpsimd.dma_start(out=t_rotp, in_=d_rotp.ap())
            nc.gpsimd.dma_start(out=t_rotk, in_=d_rotk.ap())
            nc.gpsimd.dma_start(out=t_eyek, in_=d_eyek.ap())
            nc.gpsimd.dma_start(out=t_wout, in_=d_wout.ap())
            nc.gpsimd.dma_start(out=t_eye128, in_=d_eye128.ap())
            nc.gpsimd.dma_start(out=t_trim, in_=d_trim.ap())
            nc.gpsimd.dma_start(out=t_ones, in_=d_ones.ap())

            # --- own chunk, qa first -> CC Q (q_b work for chunks 1-3 can
            # then start while the ckv gather is still in flight) ---
            # k-outer so the matmuls chase the xo/wqa DMA stream
            ps_q = [ps_tag("pss" if m < 3 else "pso", f"qo{m}") for m in range(4)]
            for k in range(KC):
                for m in range(4):
                    nc.tensor.matmul(
                        ps_q[m][:, :OWN],
                        t_wqa[:, k, m * 128 : (m + 1) * 128],
                        t_xo[:, k, :],
                        start=(k == 0),
                        stop=(k == KC - 1),
                    )
            for m in range(4):
                nc.vector.tensor_copy(t_ownq[:, m, :], ps_q[m][:, :OWN])
            nc.sync.dma_start(out=b_inQ.ap(), in_=t_ownq)
            nc.gpsimd.collective_compute(
                "AllGather", mybir.AluOpType.bypass, replica_groups=GROUPS,
                ins=[b_inQ.ap()], outs=[b_outQ.ap()],
            )

            # --- own chunk: ckv + kpe -> CC B ---
            ps_c = [ps_tag("pss" if m < 3 else "pso", f"c{m}") for m in range(4)]
            ps_k = ps_tag("av", "kpe")
            for k in range(KC):
                for m in range(4):
                    nc.tensor.matmul(
                        ps_c[m][:, :OWN],
                        t_wkva[:, k, m * 128 : (m + 1) * 128],
                        t_xo[:, k, :],
                        start=(k == 0),
                        stop=(k == KC - 1),
                    )
                nc.tensor.matmul(
                    ps_k[:64, :OWN],
                    t_wkva[:, k, 512:576],
                    t_xo[:, k, :],
                    start=(k == 0),
                    stop=(k == KC - 1),
                )
            for m in range(4):
                nc.vector.tensor_copy(t_own[:, m, :], ps_c[m][:, :OWN])
            nc.vector.tensor_copy(t_own[:64, 4, :], ps_k[:64, :OWN])
            nc.sync.dma_start(out=b_inB.ap(), in_=t_own)
            nc.gpsimd.collective_compute(
                "AllGather", mybir.AluOpType.bypass, replica_groups=GROUPS,
                ins=[b_inB.ap()], outs=[b_outB.ap()],
            )
            # qa unpack right behind CC Q (sync engine: idle once the
            # bounces are posted; t_wout lives on scalar so the CC-gated
            # unpacks can't head-of-line-block it)
            for tcb in range(1, TC):
                ts = slice(tcb * 512, (tcb + 1) * 512)
                nc.sync.dma_start(
                    out=t_qa[:, :, ts], in_=b_outQ.ap()[tcb, :, :, :]
                )

            # --- chunk 0 replicated: ckv + kpe (qa c0 comes in the qa loop) ---
            for m in range(4):
                ps = ps_any("ps3")
                for k in range(KC):
                    nc.tensor.matmul(
                        ps,
                        t_wkva[:, k, m * 128 : (m + 1) * 128],
                        t_x[:, k, :],
                        start=(k == 0),
                        stop=(k == KC - 1),
                    )
                nc.vector.tensor_copy(t_ckv[:, m, 0:512], ps)
            ps = ps_any("ps4")
            for k in range(KC):
                nc.tensor.matmul(
                    ps[:64],
                    t_wkva[:, k, 512:576],
                    t_x[:, k, :],
                    start=(k == 0),
                    stop=(k == KC - 1),
                )
            nc.vector.tensor_copy(t_kpr[:, 0:512], ps[:64])

            # --- qa chunk 0, replicated (chunks 1..3 arrive via CC A) ---
            for m in range(4):
                ps = ps_any("ps5")
                for k in range(KC):
                    nc.tensor.matmul(
                        ps,
                        t_wqa[:, k, m * 128 : (m + 1) * 128],
                        t_x[:, k, :],
                        start=(k == 0),
                        stop=(k == KC - 1),
                    )
                nc.vector.tensor_copy(t_qa[:, m, 0:512], ps)

            # --- unpack gathered ckv/kpe chunks 1..3 (chunk 0 is local) ---
            for tcb in range(1, TC):
                ts = slice(tcb * 512, (tcb + 1) * 512)
                nc.sync.dma_start(
                    out=t_ckv[:, :, ts], in_=b_outB.ap()[tcb, :, 0:4, :]
                )
                nc.sync.dma_start(
                    out=t_kpr[:, ts], in_=b_outB.ap()[tcb, 0:64, 4, :]
                )

        # ---- attention-phase persistent tiles (after ph1 frees) ----
        attn_p = st.enter_context(tc.tile_pool(name="attn_p", bufs=1))
        t_qn = attn_p.tile([128, NH, n], BF16)
        t_qpe = attn_p.tile([128, 2, n], BF16)
        t_kn = attn_p.tile([128, NH, n], BF16)
        t_kpe = attn_p.tile([128, n], BF16)
        t_v = attn_p.tile([128, NT, NH * VDIM], BF16)
        t_ao = attn_p.tile([128, NH, n], BF16)

        # ===== phase 2 (per chunk) + attention (per query block) interleaved ==
        with (
            tc.tile_pool(name="ropep", bufs=2) as rp,
            tc.tile_pool(name="ptp", bufs=5) as ptp,
            tc.tile_pool(name="smallp", bufs=2) as smallp,
            tc.tile_pool(name="otp", bufs=3) as otp,
        ):

            def rope_q(g, t):
                ts = slice(t * 512, (t + 1) * 512)
                pr = ps_any("prq")
                nc.tensor.matmul(pr, t_rotp, t_qpe[:, g, ts])
                tt1 = rp.tile([128, 512], F32, tag="tt1")
                tt2 = rp.tile([128, 512], F32, tag="tt2")
                nc.vector.tensor_tensor(tt1, pr, t_sin[:, ts], op=OP.mult)
                nc.vector.tensor_tensor(
                    tt2, t_qpe[:, g, ts], t_cos[:, ts], op=OP.mult
                )
                nc.vector.tensor_tensor(t_qpe[:, g, ts], tt1, tt2, op=OP.add)

            def rope_k(t):
                ts = slice(t * 512, (t + 1) * 512)
                pr = ps_any("prk")
                pd = ps_any("pdk")
                nc.tensor.matmul(pr, t_rotk, t_kpr[:, ts])
                nc.tensor.matmul(pd, t_eyek, t_kpr[:, ts])
                tt1 = rp.tile([128, 512], F32, tag="tt1")
                tt2 = rp.tile([128, 512], F32, tag="tt2")
                nc.vector.tensor_tensor(tt1, pr, t_sin[:, ts], op=OP.mult)
                nc.vector.tensor_tensor(tt2, pd, t_cos[:, ts], op=OP.mult)
                nc.vector.tensor_tensor(t_kpe[:, ts], tt1, tt2, op=OP.add)

            def ph2_q(t):
                ts = slice(t * 512, (t + 1) * 512)
                for m in range(6):  # q: 4 nope chunks + 2 pe chunks
                    ps = ps_any("psq")
                    for k in range(RC):
                        nc.tensor.matmul(
                            ps,
                            t_wqb[:, k, m * 128 : (m + 1) * 128],
                            t_qa[:, k, ts],
                            start=(k == 0),
                            stop=(k == RC - 1),
                        )
                    if m < 4:
                        nc.vector.tensor_copy(t_qn[:, m, ts], ps)
                    else:
                        nc.vector.tensor_copy(t_qpe[:, m - 4, ts], ps)
                rope_q(0, t)
                rope_q(1, t)
                # scatter roped q_pe into per-head zero-padded tiles
                for h in range(NH):
                    nc.vector.tensor_copy(
                        t_qpe2[0:64, h, ts],
                        t_qpe[(h % 2) * 64 : (h % 2) * 64 + 64, h // 2, ts],
                    )

            def ph2_kv(t):
                ts = slice(t * 512, (t + 1) * 512)
                for m in range(4):  # k_nope
                    ps = ps_any("psk")
                    for k in range(RC):
                        nc.tensor.matmul(
                            ps,
                            t_wk[:, k, m * 128 : (m + 1) * 128],
                            t_ckv[:, k, ts],
                            start=(k == 0),
                            stop=(k == RC - 1),
                        )
                    nc.scalar.copy(t_kn[:, m, ts], ps)
                rope_k(t)
                for mt in range(4 * t, 4 * t + 4):  # v, token-major
                    ps = ps_any("psv")
                    for k in range(RC):
                        nc.tensor.matmul(
                            ps,
                            t_ckv[:, k, mt * 128 : (mt + 1) * 128],
                            t_wv[:, k, :],
                            start=(k == 0),
                            stop=(k == RC - 1),
                        )
                    nc.scalar.copy(t_v[:, mt, :], ps)

            def attn_block(qb):
                qs = slice(qb * 512, (qb + 1) * 512)
                nkb = 4 * qb + 4
                for h in range(NH):
                    ho = (h % 2) * 64
                    g = h // 2
                    ps_av = ps_tag("av", "psav")
                    acc_v = smallp.tile([128, 512], F32, tag="accv")
                    # groups of 4 key blocks: issue all scores, then all avs,
                    # so the av->exp dependency never stalls the score stream
                    for grp in range(0, nkb, 4):
                        blocks = range(grp, min(grp + 4, nkb))
                        pts = {}
                        for kb in blocks:
                            ks = slice(kb * 128, (kb + 1) * 128)
                            diag = kb >= 4 * qb
                            r = kb - 4 * qb
                            qo = 128 * r if diag else 0
                            qsr = slice(qb * 512 + qo, (qb + 1) * 512)
                            ps_s = ps_tag("pss", "pss")
                            nc.tensor.matmul(
                                ps_s[:, qo:], t_kn[:, h, ks], t_qn[:, h, qsr],
                                start=True, stop=False,
                            )
                            if diag:
                                # in-block causal bias: -30000 where key p >
                                # query j, only the 128-wide diag band needs it
                                nc.tensor.matmul(
                                    ps_s[:, qo : qo + 128],
                                    t_eye128,
                                    t_trim,
                                    start=False, stop=False,
                                )
                            nc.tensor.matmul(
                                ps_s[:, qo:],
                                t_kpe[:, ks],
                                t_qpe2[:, h, qsr],
                                start=False, stop=True,
                            )
                            pt = ptp.tile([128, 512], BF16, tag="pt")
                            nc.scalar.activation(
                                pt[:, qo:], ps_s[:, qo:], AF.Exp, scale=SCALE
                            )
                            pts[kb] = pt
                        for kb in blocks:
                            qo = max(0, 128 * (kb - 4 * qb))
                            pt = pts[kb]
                            nc.tensor.matmul(
                                ps_av[:, qo:],
                                t_v[:, kb, h * VDIM : (h + 1) * VDIM],
                                pt[:, qo:],
                                start=(kb == 0),
                                stop=(kb == nkb - 1),
                            )
                            if kb == 0:
                                nc.vector.tensor_copy(acc_v, pt)
                            else:
                                nc.vector.tensor_tensor(
                                    acc_v[:, qo:], acc_v[:, qo:], pt[:, qo:],
                                    op=OP.add,
                                )
                    accb = smallp.tile([128, 512], BF16, tag="accb")
                    nc.scalar.copy(accb, acc_v)
                    ps_den = ps_tag("den", "psden")
                    nc.tensor.matmul(ps_den[:1], t_ones, accb)
                    rec = smallp.tile([1, 512], F32, tag="rec")
                    nc.vector.reciprocal_approx_fast(rec, ps_den[:1])
                    bc = smallp.tile([128, 512], F32, tag="bc")
                    nc.gpsimd.partition_broadcast(bc, rec)
                    nc.vector.tensor_tensor(t_ao[:, h, qs], ps_av, bc, op=OP.mult)
                # out-projection for this token block; psum banks alternate
                # pso/av (av is free here) so 4 groups are in flight
                for mt in range(4):
                    tok = qb * 512 + mt * 128
                    for f in range(4):
                        ps_o = ps_tag("pso", "pso")
                        for h4 in range(NH):
                            nc.tensor.matmul(
                                ps_o,
                                t_ao[:, h4, tok : tok + 128],
                                t_wout[:, h4, f * 512 : (f + 1) * 512],
                                start=(h4 == 0),
                                stop=(h4 == NH - 1),
                            )
                        ot = otp.tile([128, 512], BF16, tag="ot")
                        if f % 2 == 0:
                            nc.vector.tensor_copy(ot, ps_o)
                        else:
                            nc.scalar.copy(ot, ps_o)
                        # out DMAs ride the scalar ring: the sync ring has the
                        # CC-gated unpacks, which would block attn(0)'s outs
                        nc.scalar.dma_start(
                            out=d_out.ap()[tok : tok + 128, f * 512 : (f + 1) * 512],
                            in_=ot,
                        )

            # chunk 0 is fully local; q_b for chunks 1-3 needs only the (early)
            # qa gather, so it bridges the wait for the ckv gather
            ph2_q(0)
            ph2_kv(0)
            attn_block(0)
            for t in range(1, TC):
                ph2_q(t)
            for t in range(1, TC):
                ph2_kv(t)
                attn_block(t)
    return nc


_NC_CACHE = {}


def build_mla(n=2048):
    if n not in _NC_CACHE:
        nc = bacc.Bacc(
            "TRN2",
            target_bir_lowering=False,
            debug=False,
            enable_asserts=False,
        )
        _emit(nc, n)
        nc.compile()
        _NC_CACHE[n] = nc
    return _NC_CACHE[n]


def make_host_inputs(x, Wqa, Wqb, Wkva, Wkvb, Wout, n):
    """Build the 8 per-core input maps (host-side sharding)."""
    # rope tables
    theta = BASE ** (-2.0 * np.arange(ROPE // 2, dtype=np.float32) / ROPE)
    pos = np.arange(n, dtype=np.float32)
    ang = pos[:, None] * theta[None, :]  # [n, 32]
    cos64 = np.repeat(np.cos(ang).T, 2, axis=0).astype(np.float32)  # [64, n]
    sin64 = np.repeat(np.sin(ang).T, 2, axis=0).astype(np.float32)
    cosd = np.tile(cos64, (2, 1)).astype(NPBF16)  # [128, n]
    sind = np.tile(sin64, (2, 1)).astype(NPBF16)

    # in-diag-block causal bias: -30000 where key p > query j (strict)
    eye128 = np.eye(128, dtype=np.float32).astype(NPBF16)
    trimd = (
        MASK_BIAS * (np.arange(128)[:, None] > np.arange(128)[None, :])
    ).astype(NPBF16)

    rot64 = np.zeros((64, 64), np.float32)
    for i in range(32):
        rot64[2 * i + 1, 2 * i] = -1.0
        rot64[2 * i, 2 * i + 1] = 1.0
    rotp = np.zeros((128, 128), np.float32)
    rotp[:64, :64] = rot64
    rotp[64:, 64:] = rot64
    rotk = np.hstack([rot64, rot64])
    eyek = np.hstack([np.eye(64, dtype=np.float32), np.eye(64, dtype=np.float32)])

    def prelay(w, kc):
        # [kc*128, m] -> [128, kc, m] partition-major, contiguous
        return np.ascontiguousarray(
            w.reshape(kc, 128, w.shape[1]).transpose(1, 0, 2)
        ).astype(NPBF16)

    shared = {
        "wqa": prelay(Wqa, KC),
        "wkva": prelay(Wkva, KC),
        "cosd": cosd,
        "sind": sind,
        "eye128": eye128,
        "trimd": trimd,
        "onesd": np.ones((128, 1), NPBF16),

        "rotp": rotp.astype(NPBF16),
        "rotk": rotk.astype(NPBF16),
        "eyek": eyek.astype(NPBF16),
    }
    Wqb_r = Wqb.reshape(512, HEADS, Q_HEAD)
    Wkvb_r = Wkvb.reshape(KV_RANK, HEADS, NOPE + VDIM)
    Wout_r = Wout.reshape(HEADS, VDIM, EMBED)

    in_maps = []
    TC = n // 512
    # x[be].T -> [128, KC, n]: f=(c,p), tokens on free
    xT = [
        np.ascontiguousarray(
            x[be].T.reshape(KC, 128, n).transpose(1, 0, 2)
        ).astype(NPBF16)
        for be in range(x.shape[0])
    ]
    for c in range(8):
        be, hg = c // 4, c % 4
        hsel = slice(4 * hg, 4 * hg + NH)
        wqb = prelay(
            np.concatenate(
                [
                    Wqb_r[:, hsel, :NOPE].reshape(512, NH * NOPE),
                    Wqb_r[:, hsel, NOPE:].reshape(512, NH * ROPE),
                ],
                axis=1,
            ),
            RC,
        )
        in_maps.append(
            {
                **shared,
                "xc0": np.ascontiguousarray(xT[be][:, :, 0:512]),
                "xown": np.ascontiguousarray(
                    xT[be][:, :, hg * OWN : (hg + 1) * OWN]
                ),
                "wqb": wqb,
                "wk": prelay(Wkvb_r[:, hsel, :NOPE].reshape(512, NH * NOPE), RC),
                "wv": prelay(Wkvb_r[:, hsel, NOPE:].reshape(512, NH * VDIM), RC),
                "wout": prelay(Wout_r[hsel].reshape(NH * VDIM, EMBED), NH),
            }
        )
    return in_maps


def kernel(x, Wqa, Wqb, Wkva, Wkvb, Wout, _trace=False):
    x = np.asarray(x)
    b, n, _ = x.shape
    nc = build_mla(n)
    in_maps = make_host_inputs(
        np.asarray(x),
        np.asarray(Wqa),
        np.asarray(Wqb),
        np.asarray(Wkva),
        np.asarray(Wkvb),
        np.asarray(Wout),
        n,
    )
    res = bass_utils.run_bass_kernel_spmd(
        nc, in_maps, core_ids=list(range(8)), trace=_trace
    )
    out = np.zeros((b, n, EMBED), np.float32)
    for c in range(8):
        out[c // 4] += np.asarray(res.results[c]["out"]).astype(np.float32)
    if _trace:
        kernel.last_results = res
    return out

